# revision 1
# baseline (speedup 1.0000x reference)
"""Trainium2 Bass kernel for nn_Causal_Kron_Block_MLP.

Reference computation (B=4, L=2048, D=1024, H=16, HD=64):
    y1 = x @ W1a.T                                   # [B,L,D]
    z  = relu(einsum('hlm,bhmd->bhld', tril(mat2a), split_heads(y1)))
    y2 = merge_heads(z) @ W1b.T
    w  = einsum('hlm,bhmd->bhld', tril(mat2b), split_heads(y2))
    out = einsum('bhld,hde->ble', w, w_out)

Sharding: 8 cores, head-parallel — core c owns heads (2c, 2c+1).
Each core computes y1/z for its 2 heads over the full batch; an
AllGather (split in two chunks, overlapped with compute) exchanges z
(the only cross-head mixing point is W1b); each core then computes
the y2 columns for its heads, the tril_b stage, and a partial
head-sum of the output; the host sums the 8 partials.

Layouts (device, per core; r = global row index (b, l), R = 8192):
    y1mT/y2mT: per (h_rel, p, m-block) tiles [128 = m, 128 = (j, d)]
               built by PE-transposes fused with stages 1/3
    z_all0/1:  [512 = (rank, d), R] per h_rel chunk (AllGather out)
    wT_sb:     [128 = (h_rel, d), R]  (stage-4 output, reassembled)
    out_part:  [R, D] fp16, scaled by 1024 (values ~1e-5 would be
               fp16-subnormal unscaled); the host sums in f32 and
               rescales.

All matmuls run in fp16 (1 PE cycle/row, 10 mantissa bits; inputs
pre-cast on the host, intermediates rounded by the PSUM->SBUF copies)
with f32 PSUM accumulation; measured end-to-end relative error vs the
f32 reference is ~1e-3. Causality: tril blocks entirely above the
diagonal are never loaded nor multiplied; diagonal blocks skip their
zero prefix. DMAs are batched via multi-dim access patterns.
"""

import numpy as np

import concourse.bass as bass
import concourse.mybir as mybir
import concourse.tile as tile
from concourse import bacc
from concourse.bass_utils import run_bass_kernel_spmd

B, L, D, H, HD = 4, 2048, 1024, 16, 64
NCORES = 8
R = B * L               # 8192 global rows
NB = 512                # moving free-dim per matmul
N_RB = R // NB          # 16 row-blocks of 512
N_KB_D = D // 128       # 8 k-blocks over model dim
N_MB = L // 128         # 16 m-blocks over seq per batch
N_LB = L // NB          # 4 l-blocks of 512 per batch
MB_G = 4                # tril m-blocks fetched per DMA
OUT_SCALE = 1024.0
F32 = mybir.dt.float32
FP16 = mybir.dt.float16

_NC_CACHE = {}


def build_nc():
    """Build the single-NEFF SPMD kernel (same program on all 8 cores)."""
    nc = bacc.Bacc(None, target_bir_lowering=False)

    xT = nc.dram_tensor("xT", [D, R], FP16, kind="ExternalInput")
    w1aT = nc.dram_tensor("w1aT", [D, 128], FP16, kind="ExternalInput")
    # w1bT rows are host-permuted to the chunked-AllGather k order:
    # chunk h_rel, then (rank, d).
    w1bT = nc.dram_tensor("w1bT", [D, 128], FP16, kind="ExternalInput")
    trilAT = nc.dram_tensor("trilAT", [2, L, L], FP16, kind="ExternalInput")
    trilBT = nc.dram_tensor("trilBT", [2, L, L], FP16, kind="ExternalInput")
    wout = nc.dram_tensor("wout", [128, D], FP16, kind="ExternalInput")
    ident_in = nc.dram_tensor("ident", [128, 128], FP16, kind="ExternalInput")
    out_part = nc.dram_tensor("out_part", [R, D], FP16, kind="ExternalOutput")

    with tile.TileContext(nc) as tc:
        with (
            tc.tile_pool(name="persist", bufs=1) as persist,
            tc.tile_pool(name="stg0", bufs=3) as stg0,
            tc.tile_pool(name="psmm", bufs=5, space="PSUM") as psmm,
            tc.tile_pool(name="pstr", bufs=3, space="PSUM") as pstr,
            tc.tile_pool(name="dram", bufs=1, space="DRAM") as dram,
        ):
            ident = persist.tile([128, 128], FP16, tag="ident")
            nc.sync.dma_start(out=ident[:], in_=ident_in[:])

            w1aT_sb = persist.tile([128, D], FP16, tag="w1aT")
            w1bT_sb = persist.tile([128, D], FP16, tag="w1bT")
            wout_sb = persist.tile([128, D], FP16, tag="wout")
            nc.sync.dma_start(
                out=w1aT_sb[:].rearrange("p (g n) -> p g n", g=N_KB_D),
                in_=w1aT[:].rearrange("(g p) n -> p g n", p=128),
            )
            nc.sync.dma_start(
                out=w1bT_sb[:].rearrange("p (g n) -> p g n", g=N_KB_D),
                in_=w1bT[:].rearrange("(g p) n -> p g n", p=128),
            )
            nc.sync.dma_start(out=wout_sb[:], in_=wout[:])

            # AllGather chunks: z_in[h_rel] [HD, R] -> z_all[h_rel] [8*HD, R]
            z_in = [
                dram.tile([HD, R], FP16, tag=f"z_in{h}", name=f"z_in{h}")
                for h in range(2)
            ]
            z_all = [
                dram.tile(
                    [NCORES * HD, R], FP16, tag=f"z_all{h}",
                    name=f"z_all{h}", addr_space="Shared",
                )
                for h in range(2)
            ]

            # -------- stage 1/3 + fused transpose glue ------------------
            def linear_stage(src_loads, wT_sb_, dstmT, scope, rb_order=None,
                             split_chains=False):
                """dstmT tiles [128=m, 128=(j,d)] per (h_rel, p, mb) from
                out[128=(h_rel,d), r] = wT_sb_.T @ src, PE-transposed.
                src_loads(rb) -> list of (tile, kb_lo, kb_hi).
                split_chains: one PSUM chain per source group (so a group's
                matmuls don't wait on later groups' inputs), summed after."""
                with nc.named_scope(scope):
                    for rb in (rb_order or range(N_RB)):
                        b, lc = rb // N_LB, rb % N_LB
                        p, j = b // 2, b % 2
                        groups = src_loads(rb)
                        yt = persist.tile(
                            [128, NB], FP16, tag="yt", bufs=6,
                            name=f"yt_{scope}_{rb}",
                        )
                        if not split_chains or len(groups) == 1:
                            ps = psmm.tile([128, NB], F32, tag="ps_mm")
                            for src, kb_lo, kb_hi in groups:
                                for kb in range(kb_lo, kb_hi):
                                    nc.tensor.matmul(
                                        ps[:],
                                        wT_sb_[:, kb * 128 : (kb + 1) * 128],
                                        src[
                                            :,
                                            (kb - kb_lo) * NB
                                            : (kb - kb_lo + 1) * NB,
                                        ],
                                        start=(kb == 0),
                                        stop=(kb == N_KB_D - 1),
                                    )
                            nc.scalar.activation(
                                yt[:], ps[:], mybir.ActivationFunctionType.Copy
                            )
                        else:
                            ps_list = []
                            for src, kb_lo, kb_hi in groups:
                                ps = psmm.tile(
                                    [128, NB], F32, tag="ps_mm",
                                    name=f"ps_{scope}_{rb}_{kb_lo}",
                                )
                                for kb in range(kb_lo, kb_hi):
                                    nc.tensor.matmul(
                                        ps[:],
                                        wT_sb_[:, kb * 128 : (kb + 1) * 128],
                                        src[
                                            :,
                                            (kb - kb_lo) * NB
                                            : (kb - kb_lo + 1) * NB,
                                        ],
                                        start=(kb == kb_lo),
                                        stop=(kb == kb_hi - 1),
                                    )
                                ps_list.append(ps)
                            tmp = stg0.tile([128, NB], F32, tag="ysum",
                                            name=f"ys_{scope}_{rb}")
                            nc.scalar.activation(
                                tmp[:], ps_list[0][:],
                                mybir.ActivationFunctionType.Copy,
                            )
                            nc.vector.tensor_tensor(
                                yt[:], tmp[:], ps_list[1][:],
                                mybir.AluOpType.add,
                            )
                        # one 128-wide transpose covers both h_rel halves
                        for ml in range(NB // 128):
                            mb = lc * (NB // 128) + ml
                            pst = pstr.tile([128, 128], FP16, tag="ps_tr")
                            nc.tensor.transpose(
                                pst[:],
                                yt[:, ml * 128 : (ml + 1) * 128],
                                ident[:],
                            )
                            for h_rel in range(2):
                                off = ((h_rel * 2 + p) * N_MB + mb) * 128
                                nc.vector.tensor_copy(
                                    dstmT[:, off + j * HD : off + (j + 1) * HD],
                                    pst[:, h_rel * HD : (h_rel + 1) * HD],
                                )

            # -------- stages 2/4: out = y.T @ trilT (causal) ------------
            def tril_stage(trilT, srcmT, out_cb, tpool, scope, hl_list):
                # Full 512-wide m-block groups strictly below the diagonal,
                # then 4 diagonal m-blocks loaded without their zero prefix.
                with nc.named_scope(scope):
                    for h_rel, lb in hl_list:
                        n_mb = (lb + 1) * (NB // 128)
                        pss = [
                            psmm.tile(
                                [128, NB], F32, tag="ps_mm",
                                name=f"ps_{scope}_{h_rel}_{lb}_{pi}",
                            )
                            for pi in range(2)
                        ]
                        for mg in range(0, lb * MB_G, MB_G):
                            tblk = tpool.tile(
                                [128, MB_G * NB], FP16, tag="tril_blk",
                                name=f"tb_{scope}_{h_rel}_{lb}_{mg}",
                            )
                            nc.sync.dma_start(
                                out=tblk[:].rearrange(
                                    "p (g n) -> p g n", g=MB_G
                                ),
                                in_=trilT[
                                    h_rel,
                                    mg * 128 : (mg + MB_G) * 128,
                                    lb * NB : (lb + 1) * NB,
                                ].rearrange("(g p) n -> p g n", p=128),
                            )
                            for mi in range(MB_G):
                                mb = mg + mi
                                for p in range(2):
                                    off = ((h_rel * 2 + p) * N_MB + mb) * 128
                                    nc.tensor.matmul(
                                        pss[p][:],
                                        srcmT[:, off : off + 128],
                                        tblk[:, mi * NB : (mi + 1) * NB],
                                        start=(mb == 0),
                                        stop=False,
                                    )
                        # diagonal group: m-block lb*4+i has i*128 leading zeros
                        for i in range(MB_G):
                            mb = lb * MB_G + i
                            w = NB - i * 128
                            dblk = tpool.tile(
                                [128, NB], FP16, tag="diag_blk",
                                name=f"db_{scope}_{h_rel}_{lb}_{i}",
                            )
                            nc.sync.dma_start(
                                out=dblk[:, :w],
                                in_=trilT[
                                    h_rel,
                                    mb * 128 : (mb + 1) * 128,
                                    lb * NB + i * 128 : (lb + 1) * NB,
                                ],
                            )
                            for p in range(2):
                                off = ((h_rel * 2 + p) * N_MB + mb) * 128
                                nc.tensor.matmul(
                                    pss[p][:, i * 128 : NB],
                                    srcmT[:, off : off + 128],
                                    dblk[:, :w],
                                    start=(mb == 0),
                                    stop=(i == MB_G - 1),
                                )
                        for p in range(2):
                            out_cb(h_rel, p, lb, pss[p])

            # ================= phase A ==================================
            with (
                tc.tile_pool(name="xin", bufs=3) as xin,
                tc.tile_pool(name="trilA_p", bufs=5) as trilA_p,
                tc.tile_pool(name="mtA", bufs=1) as mtA,
            ):
                y1mT = mtA.tile([128, 2 * R], FP16, tag="y1mT")
                z_sb = mtA.tile([128, R], FP16, tag="z_sb")

                def x_load(rb):
                    xt = xin.tile([128, N_KB_D * NB], FP16, tag="x_blk",
                                  name=f"x_{rb}")
                    nc.sync.dma_start(
                        out=xt[:].rearrange("p (g n) -> p g n", g=N_KB_D),
                        in_=xT[:, rb * NB : (rb + 1) * NB].rearrange(
                            "(g p) n -> p g n", p=128
                        ),
                    )
                    return [(xt, 0, N_KB_D)]

                linear_stage(x_load, w1aT_sb, y1mT, "s1")

                def z_out(h_rel, p, lb, ps):
                    base = (h_rel * 2 + p) * L
                    nc.scalar.activation(
                        z_sb[:, base + lb * NB : base + (lb + 1) * NB],
                        ps[:],
                        mybir.ActivationFunctionType.Relu,
                    )

                # per-h_rel: stage 2 chunk -> z_in DMAs -> AllGather, so the
                # first gather overlaps the second chunk's compute
                for h_rel in range(2):
                    tril_stage(trilAT, y1mT, z_out, trilA_p, f"s2h{h_rel}",
                               [(h_rel, lb) for lb in range(N_LB)])
                    with nc.named_scope(f"ag_in{h_rel}"):
                        for p in range(2):
                            for j in range(2):
                                b = 2 * p + j
                                nc.sync.dma_start(
                                    out=z_in[h_rel][:, b * L : (b + 1) * L],
                                    in_=z_sb[
                                        j * HD : (j + 1) * HD,
                                        (h_rel * 2 + p) * L
                                        : (h_rel * 2 + p + 1) * L,
                                    ],
                                )
                    nc.gpsimd.collective_compute(
                        "AllGather",
                        mybir.AluOpType.bypass,
                        replica_groups=[list(range(NCORES))],
                        ins=[z_in[h_rel].opt()],
                        outs=[z_all[h_rel].opt()],
                    )

            # ================= phase B ==================================
            with (
                tc.tile_pool(name="zin_p", bufs=3) as zin_p,
                tc.tile_pool(name="trilB_p", bufs=5) as trilB_p,
                tc.tile_pool(name="mtB", bufs=1) as mtB,
                tc.tile_pool(name="stg", bufs=3) as stg,
            ):
                y2mT = mtB.tile([128, 2 * R], FP16, tag="y2mT")
                wT_sb = mtB.tile([128, R], FP16, tag="wT_sb")

                def z_load(rb):
                    # separate tiles per AG chunk so chunk-0 matmuls don't
                    # wait for the second AllGather
                    out = []
                    for h_rel in range(2):
                        zt = zin_p.tile(
                            [128, 4 * NB], FP16, tag=f"z_blk{h_rel}",
                            name=f"z_{h_rel}_{rb}",
                        )
                        nc.sync.dma_start(
                            out=zt[:].rearrange("p (g n) -> p g n", g=4),
                            in_=z_all[h_rel][
                                :, rb * NB : (rb + 1) * NB
                            ].rearrange("(g p) n -> p g n", p=128),
                        )
                        out.append((zt, h_rel * 4, h_rel * 4 + 4))
                    return out

                linear_stage(
                    z_load, w1bT_sb, y2mT, "s3",
                    rb_order=[b * N_LB + lc for lc in range(N_LB)
                              for b in range(B)],
                    split_chains=True,
                )

                def w_cb(h_rel, p, lb, ps):
                    st = stg.tile([128, NB], FP16, tag="w_stage",
                                  name=f"wst_{h_rel}_{p}_{lb}")
                    nc.scalar.activation(
                        st[:], ps[:], mybir.ActivationFunctionType.Copy
                    )
                    for j in range(2):
                        b = 2 * p + j
                        nc.sync.dma_start(
                            out=wT_sb[
                                h_rel * HD : (h_rel + 1) * HD,
                                b * L + lb * NB : b * L + (lb + 1) * NB,
                            ],
                            in_=st[j * HD : (j + 1) * HD, :],
                        )

                tril_stage(
                    trilBT, y2mT, w_cb, trilB_p, "s4",
                    [(h_rel, lb) for lb in range(N_LB) for h_rel in range(2)],
                )

                # stage 5: out_part rows = (wT.T @ wout) * OUT_SCALE
                with nc.named_scope("s5"):
                    for rb in range(R // 128):
                        ost = stg.tile([128, D], FP16, tag="out_stage",
                                       bufs=3, name=f"ost_{rb}")
                        for eh in range(2):
                            ps = psmm.tile([128, NB], F32, tag="ps_mm",
                                           name=f"ps5_{rb}_{eh}")
                            nc.tensor.matmul(
                                ps[:],
                                wT_sb[:, rb * 128 : (rb + 1) * 128],
                                wout_sb[:, eh * NB : (eh + 1) * NB],
                                start=True,
                                stop=True,
                            )
                            nc.scalar.activation(
                                ost[:, eh * NB : (eh + 1) * NB],
                                ps[:],
                                mybir.ActivationFunctionType.Copy,
                                scale=OUT_SCALE,
                            )
                        nc.sync.dma_start(
                            out=out_part[rb * 128 : (rb + 1) * 128, :],
                            in_=ost[:],
                        )

    nc.finalize()
    return nc


def prep_in_maps(x, W1a, W1b, mat2a, mat2b, w_out):
    xT = np.ascontiguousarray(x.reshape(R, D).T).astype(np.float16)
    ident = np.eye(128, dtype=np.float16)
    # chunked-AG k order: (h_rel, rank, d) -> head h = 2*rank + h_rel
    k_perm = np.array(
        [2 * rank + h_rel for h_rel in range(2) for rank in range(NCORES)]
    )
    in_maps = []
    for c in range(NCORES):
        heads = [2 * c, 2 * c + 1]
        W1b_c = W1b[128 * c : 128 * (c + 1), :]  # [128 out-cols, D]
        W1b_c_perm = (
            W1b_c.reshape(128, H, HD)[:, k_perm, :].reshape(128, D)
        )
        in_maps.append(
            {
                "xT": xT,
                "w1aT": np.ascontiguousarray(
                    W1a[128 * c : 128 * (c + 1), :].T
                ).astype(np.float16),
                "w1bT": np.ascontiguousarray(W1b_c_perm.T).astype(np.float16),
                "trilAT": np.stack(
                    [np.tril(mat2a[h]).T.astype(np.float16) for h in heads]
                ),
                "trilBT": np.stack(
                    [np.tril(mat2b[h]).T.astype(np.float16) for h in heads]
                ),
                "wout": w_out[heads].reshape(128, D).astype(np.float16),
                "ident": ident,
            }
        )
    return in_maps


def kernel(x, W1a, W1b, mat2a, mat2b, w_out):
    x = np.asarray(x, dtype=np.float32)
    W1a = np.asarray(W1a, dtype=np.float32)
    W1b = np.asarray(W1b, dtype=np.float32)
    mat2a = np.asarray(mat2a, dtype=np.float32)
    mat2b = np.asarray(mat2b, dtype=np.float32)
    w_out = np.asarray(w_out, dtype=np.float32)

    if "nc" not in _NC_CACHE:
        _NC_CACHE["nc"] = build_nc()
    nc = _NC_CACHE["nc"]

    in_maps = prep_in_maps(x, W1a, W1b, mat2a, mat2b, w_out)
    res = run_bass_kernel_spmd(nc, in_maps, core_ids=list(range(NCORES)))
    out = np.zeros((R, D), np.float32)
    for c in range(NCORES):
        out += res.results[c]["out_part"].astype(np.float32)
    out *= 1.0 / OUT_SCALE
    return out.reshape(B, L, D)


if __name__ == "__main__":
    rng = np.random.default_rng(0)
    inputs = {
        "x": rng.standard_normal((B, L, D), dtype=np.float32),
        "W1a": rng.standard_normal((D, D), dtype=np.float32) / D,
        "W1b": rng.standard_normal((D, D), dtype=np.float32) / D,
        "mat2a": rng.standard_normal((H, L, L), dtype=np.float32) / 32,
        "mat2b": rng.standard_normal((H, L, L), dtype=np.float32) / 32,
        "w_out": rng.standard_normal((H, HD, D), dtype=np.float32) / D,
    }
    out = kernel(**inputs)
    print("kernel ran, out shape", out.shape)



# revision 2
# speedup vs baseline: 1.0459x; 1.0459x over previous
"""Trainium2 Bass kernel for nn_Causal_Kron_Block_MLP.

Reference computation (B=4, L=2048, D=1024, H=16, HD=64):
    y1 = x @ W1a.T                                   # [B,L,D]
    z  = relu(einsum('hlm,bhmd->bhld', tril(mat2a), split_heads(y1)))
    y2 = merge_heads(z) @ W1b.T
    w  = einsum('hlm,bhmd->bhld', tril(mat2b), split_heads(y2))
    out = einsum('bhld,hde->ble', w, w_out)

Sharding: 8 cores, head-parallel — core c owns heads (2c, 2c+1).
The kernel is one software pipeline ordered by sequence chunk
(lc = 512-row l-blocks):

  phase A, per lc: s1 (x @ W1a.T for the 4 batches' lc rows, with
    fused PE transpose) -> s2 (causal tril_a chunk, all m <= lc) ->
    relu -> AllGather chunk lc (z for all heads, rows (b, lc)).
    The 4 chunked AllGathers overlap later chunks' compute.
  phase B, per lc: s3 (W1b over the gathered z chunk) -> s4 (tril_b)
    -> stage rows into AllToAll buffers.  Two AllToAlls redistribute
    w from head-parallel to row-parallel layout (core c ends with all
    16 heads for global rows [c*1024, (c+1)*1024)); each fires as
    soon as its two quarter-chunks of s4 are done.
  s5: out rows = (w_all.T @ wout_all) * OUT_SCALE for the core's 1024
    rows only; the host concatenates (no reduction).

All heavy DMA sources are host-pre-tiled so every descriptor moves
>=1KB contiguous runs per partition (x tiles 8KB, tril full-blocks
4KB).  Causality: tril blocks above the diagonal are never loaded;
diagonal blocks are packed host-side with their zero prefix stripped.
All matmuls run in fp16 with f32 PSUM accumulation; measured
end-to-end relative error vs the f32 reference is ~1e-3.  out_part is
fp16 scaled by 1024 (values ~1e-5 would be fp16-subnormal unscaled);
the host rescales in f32.
"""

import numpy as np

import concourse.bass as bass
import concourse.mybir as mybir
import concourse.tile as tile
from concourse import bacc
from concourse.bass_utils import run_bass_kernel_spmd

B, L, D, H, HD = 4, 2048, 1024, 16, 64
NCORES = 8
R = B * L               # 8192 global rows
NB = 512                # moving free-dim per matmul
N_RB = R // NB          # 16 row-blocks of 512
N_KB_D = D // 128       # 8 k-blocks over model dim
N_MB = L // 128         # 16 m-blocks of 128 per batch
N_LB = L // NB          # 4 l-blocks of 512 per batch
FG_BASE = {1: 0, 2: 1, 3: 3}   # lb -> first full-group index (lb groups)
N_FG = 6                # total full 4x128-m-block groups per h_rel
COL_OFF = [0, 512, 896, 1152]  # packed diag block col offsets (w=512..128)
OUT_SCALE = 1024.0
F32 = mybir.dt.float32
FP16 = mybir.dt.float16

_NC_CACHE = {}


def build_nc():
    """Build the single-NEFF SPMD kernel (same program on all 8 cores)."""
    nc = bacc.Bacc(None, target_bir_lowering=False)

    xT_t = nc.dram_tensor("xT_t", [N_RB, 128, N_KB_D * NB], FP16,
                          kind="ExternalInput")
    w1aT = nc.dram_tensor("w1aT", [128, D], FP16, kind="ExternalInput")
    w1bT = nc.dram_tensor("w1bT", [128, D], FP16, kind="ExternalInput")
    trilAF = nc.dram_tensor("trilAF", [2, N_FG, 128, 4 * NB], FP16,
                            kind="ExternalInput")
    trilAD = nc.dram_tensor("trilAD", [2, N_LB, 128, 1280], FP16,
                            kind="ExternalInput")
    trilBF = nc.dram_tensor("trilBF", [2, N_FG, 128, 4 * NB], FP16,
                            kind="ExternalInput")
    trilBD = nc.dram_tensor("trilBD", [2, N_LB, 128, 1280], FP16,
                            kind="ExternalInput")
    wout_t = nc.dram_tensor("wout_t", [128, NCORES * D], FP16,
                            kind="ExternalInput")
    ident_in = nc.dram_tensor("ident", [128, 128], FP16, kind="ExternalInput")
    out_part = nc.dram_tensor("out_part", [R // NCORES, D], FP16,
                              kind="ExternalOutput")

    with tile.TileContext(nc) as tc:
        with (
            tc.tile_pool(name="persist", bufs=1) as persist,
            tc.tile_pool(name="xin", bufs=2) as xin,
            tc.tile_pool(name="tfa", bufs=2) as tfa,
            tc.tile_pool(name="tda", bufs=2) as tda,
            tc.tile_pool(name="tfb", bufs=2) as tfb,
            tc.tile_pool(name="tdb", bufs=2) as tdb,
            tc.tile_pool(name="zap", bufs=2) as zap,
            tc.tile_pool(name="a2ap", bufs=2) as a2ap,
            tc.tile_pool(name="ytp", bufs=4) as ytp,
            tc.tile_pool(name="stp", bufs=6) as stp,
            tc.tile_pool(name="ostp", bufs=2) as ostp,
            tc.tile_pool(name="psmm", bufs=5, space="PSUM") as psmm,
            tc.tile_pool(name="pstr", bufs=3, space="PSUM") as pstr,
            tc.tile_pool(name="dram", bufs=1, space="DRAM") as dram,
        ):
            ident = persist.tile([128, 128], FP16, tag="ident")
            nc.sync.dma_start(out=ident[:], in_=ident_in[:])
            w1aT_sb = persist.tile([128, D], FP16, tag="w1aT")
            nc.sync.dma_start(out=w1aT_sb[:], in_=w1aT[:])
            w1bT_sb = persist.tile([128, D], FP16, tag="w1bT")
            wout_sb = persist.tile([128, NCORES * D], FP16, tag="wout")

            y1mT = persist.tile([128, 2 * R], FP16, tag="y1mT")
            y2mT = persist.tile([128, 2 * R], FP16, tag="y2mT")

            z_in = [
                dram.tile([128, B * NB], FP16, tag=f"z_in{lc}",
                          name=f"z_in{lc}")
                for lc in range(N_LB)
            ]
            z_all = [
                dram.tile([NCORES * 128, B * NB], FP16, tag=f"z_all{lc}",
                          name=f"z_all{lc}", addr_space="Shared")
                for lc in range(N_LB)
            ]
            a2a_in = [
                dram.tile([NCORES * 128, NB], FP16, tag=f"a2a_in{q}",
                          name=f"a2a_in{q}")
                for q in range(2)
            ]
            a2a_out = [
                dram.tile([NCORES * 128, NB], FP16, tag=f"a2a_out{q}",
                          name=f"a2a_out{q}")
                for q in range(2)
            ]

            def lin_rb(scope, rb, wsb, src_tile, dstmT):
                """One 512-row block of stage 1/3 with fused PE transpose:
                dstmT[(h_rel,p,mb) 128-col blocks][m-part, (j,d)]."""
                b, lc = rb // N_LB, rb % N_LB
                p, j = b // 2, b % 2
                ps = psmm.tile([128, NB], F32, tag="ps_mm",
                               name=f"ps_{scope}_{rb}")
                for kb in range(N_KB_D):
                    nc.tensor.matmul(
                        ps[:],
                        wsb[:, kb * 128:(kb + 1) * 128],
                        src_tile[:, kb * NB:(kb + 1) * NB],
                        start=(kb == 0),
                        stop=(kb == N_KB_D - 1),
                    )
                yt = ytp.tile([128, NB], FP16, tag="yt",
                              name=f"yt_{scope}_{rb}")
                nc.scalar.activation(
                    yt[:], ps[:], mybir.ActivationFunctionType.Copy
                )
                for ml in range(NB // 128):
                    mb = lc * (NB // 128) + ml
                    pst = pstr.tile([128, 128], FP16, tag="ps_tr")
                    nc.tensor.transpose(
                        pst[:], yt[:, ml * 128:(ml + 1) * 128], ident[:]
                    )
                    for h_rel in range(2):
                        off = ((h_rel * 2 + p) * N_MB + mb) * 128
                        nc.vector.tensor_copy(
                            dstmT[:, off + j * HD: off + (j + 1) * HD],
                            pst[:, h_rel * HD:(h_rel + 1) * HD],
                        )

            def tril_block(scope, trilF, trilD, fpool, dpool, srcmT,
                           h_rel, lb, drain):
                """Stage 2/4 chunk: z.T[(j,d), l-cols of lb] for one head,
                accumulating over all m-blocks <= diag (causal)."""
                pss = [
                    psmm.tile([128, NB], F32, tag="ps_mm",
                              name=f"ps_{scope}_{h_rel}_{lb}_{p}")
                    for p in range(2)
                ]
                for g in range(lb):
                    tb = fpool.tile([128, 4 * NB], FP16, tag="tf",
                                    name=f"tf_{scope}_{h_rel}_{lb}_{g}")
                    nc.sync.dma_start(out=tb[:],
                                      in_=trilF[h_rel, FG_BASE[lb] + g])
                    for mi in range(4):
                        mb = 4 * g + mi
                        for p in range(2):
                            off = ((h_rel * 2 + p) * N_MB + mb) * 128
                            nc.tensor.matmul(
                                pss[p][:],
                                srcmT[:, off:off + 128],
                                tb[:, mi * NB:(mi + 1) * NB],
                                start=(mb == 0),
                                stop=False,
                            )
                td = dpool.tile([128, 1280], FP16, tag="td",
                                name=f"td_{scope}_{h_rel}_{lb}")
                nc.sync.dma_start(out=td[:], in_=trilD[h_rel, lb])
                for i in range(4):
                    mb = lb * 4 + i
                    w = NB - i * 128
                    for p in range(2):
                        off = ((h_rel * 2 + p) * N_MB + mb) * 128
                        nc.tensor.matmul(
                            pss[p][:, i * 128:NB],
                            srcmT[:, off:off + 128],
                            td[:, COL_OFF[i]:COL_OFF[i] + w],
                            start=(mb == 0),
                            stop=(i == 3),
                        )
                for p in range(2):
                    drain(h_rel, p, lb, pss[p])

            # ================= phase A ==================================
            def z_drain(h_rel, p, lb, ps):
                st = stp.tile([128, NB], FP16, tag="zst",
                              name=f"zst_{h_rel}_{p}_{lb}")
                nc.scalar.activation(
                    st[:], ps[:], mybir.ActivationFunctionType.Relu
                )
                for jb in range(2):
                    bb = 2 * p + jb
                    nc.sync.dma_start(
                        out=z_in[lb][h_rel * HD:(h_rel + 1) * HD,
                                     bb * NB:(bb + 1) * NB],
                        in_=st[jb * HD:(jb + 1) * HD, :],
                    )

            for lc in range(N_LB):
                with nc.named_scope(f"s1c{lc}"):
                    for b in range(B):
                        rb = b * N_LB + lc
                        xt = xin.tile([128, N_KB_D * NB], FP16, tag="x_blk",
                                      name=f"x_{rb}")
                        nc.sync.dma_start(out=xt[:], in_=xT_t[rb])
                        lin_rb("s1", rb, w1aT_sb, xt, y1mT)
                with nc.named_scope(f"s2c{lc}"):
                    for h_rel in range(2):
                        tril_block("s2", trilAF, trilAD, tfa, tda, y1mT,
                                   h_rel, lc, z_drain)
                nc.gpsimd.collective_compute(
                    "AllGather",
                    mybir.AluOpType.bypass,
                    replica_groups=[list(range(NCORES))],
                    ins=[z_in[lc].opt()],
                    outs=[z_all[lc].opt()],
                )
                if lc == 0:
                    # phase-B weights: emitted here so the DMAs land during
                    # phase A's slack, well before s3/s5 need them
                    nc.sync.dma_start(out=w1bT_sb[:], in_=w1bT[:])
                    nc.sync.dma_start(out=wout_sb[:], in_=wout_t[:])

            # ================= phase B ==================================
            def w_drain(h_rel, p, lb, ps):
                st = stp.tile([128, NB], FP16, tag="wst",
                              name=f"wst_{h_rel}_{p}_{lb}")
                nc.scalar.activation(
                    st[:], ps[:], mybir.ActivationFunctionType.Copy
                )
                q = lb % 2
                for jb in range(2):
                    bb = 2 * p + jb
                    dest = bb * 2 + lb // 2
                    nc.sync.dma_start(
                        out=a2a_in[q][dest * 128 + h_rel * HD:
                                      dest * 128 + (h_rel + 1) * HD, :],
                        in_=st[jb * HD:(jb + 1) * HD, :],
                    )

            for lc in range(N_LB):
                with nc.named_scope(f"s3c{lc}"):
                    for b in range(B):
                        rb = b * N_LB + lc
                        zt = zap.tile([128, N_KB_D * NB], FP16, tag="z_blk",
                                      name=f"z3_{rb}")
                        nc.sync.dma_start(
                            out=zt[:].rearrange("p (g n) -> p g n",
                                                g=N_KB_D),
                            in_=z_all[lc][:, b * NB:(b + 1) * NB].rearrange(
                                "(g p) n -> p g n", p=128),
                        )
                        lin_rb("s3", rb, w1bT_sb, zt, y2mT)
                with nc.named_scope(f"s4c{lc}"):
                    for h_rel in range(2):
                        tril_block("s4", trilBF, trilBD, tfb, tdb, y2mT,
                                   h_rel, lc, w_drain)
                # A2A q covers dest quarters lc%2: needs s4 of lc and lc+2
                if lc >= 2:
                    nc.gpsimd.collective_compute(
                        "AllToAll",
                        mybir.AluOpType.bypass,
                        replica_groups=[list(range(NCORES))],
                        ins=[a2a_in[lc % 2].opt()],
                        outs=[a2a_out[lc % 2].opt()],
                    )

            # ================= stage 5 ==================================
            with nc.named_scope("s5"):
                for q in range(2):
                    wt = a2ap.tile([128, N_KB_D * NB], FP16, tag="a2a_sb",
                                   name=f"a2a_{q}")
                    nc.sync.dma_start(
                        out=wt[:].rearrange("p (g n) -> p g n", g=N_KB_D),
                        in_=a2a_out[q][:].rearrange("(g p) n -> p g n",
                                                    p=128),
                    )
                    for rblk in range(NB // 128):
                        ost = ostp.tile([128, D], FP16, tag="out_stage",
                                        name=f"ost_{q}_{rblk}")
                        for eh in range(2):
                            ps = psmm.tile([128, NB], F32, tag="ps_mm",
                                           name=f"ps5_{q}_{rblk}_{eh}")
                            for kb in range(N_KB_D):
                                nc.tensor.matmul(
                                    ps[:],
                                    wt[:, kb * NB + rblk * 128:
                                       kb * NB + (rblk + 1) * 128],
                                    wout_sb[:, kb * D + eh * NB:
                                            kb * D + (eh + 1) * NB],
                                    start=(kb == 0),
                                    stop=(kb == N_KB_D - 1),
                                )
                            nc.scalar.activation(
                                ost[:, eh * NB:(eh + 1) * NB],
                                ps[:],
                                mybir.ActivationFunctionType.Copy,
                                scale=OUT_SCALE,
                            )
                        nc.sync.dma_start(
                            out=out_part[q * NB + rblk * 128:
                                         q * NB + (rblk + 1) * 128, :],
                            in_=ost[:],
                        )

    nc.finalize()
    return nc


def _tril_tiles(mat_h):
    """Host pre-tiling of one head's tril matrix (transposed, fp16):
    full groups [N_FG//?][128, 4*NB] and packed diag [N_LB][128, 1280]."""
    T = np.tril(mat_h).T.astype(np.float16)      # [L, L], upper (m <= l)
    F = np.zeros((N_FG, 128, 4 * NB), np.float16)
    for lb in range(1, N_LB):
        for g in range(lb):
            blk = T[g * NB:(g + 1) * NB, lb * NB:(lb + 1) * NB]
            F[FG_BASE[lb] + g] = (
                blk.reshape(4, 128, NB).transpose(1, 0, 2).reshape(128, 4 * NB)
            )
    Dg = np.zeros((N_LB, 128, 1280), np.float16)
    for lb in range(N_LB):
        for i in range(4):
            mb = lb * 4 + i
            w = NB - i * 128
            Dg[lb][:, COL_OFF[i]:COL_OFF[i] + w] = T[
                mb * 128:(mb + 1) * 128, lb * NB + i * 128:(lb + 1) * NB
            ]
    return F, Dg


def prep_in_maps(x, W1a, W1b, mat2a, mat2b, w_out):
    xT = np.ascontiguousarray(x.reshape(R, D).T).astype(np.float16)
    xT_t = np.ascontiguousarray(
        xT.reshape(N_KB_D, 128, N_RB, NB).transpose(2, 1, 0, 3)
    ).reshape(N_RB, 128, N_KB_D * NB)
    ident = np.eye(128, dtype=np.float16)
    # k order for the gathered z / exchanged w: (rank, h_rel, d)
    k_idx = np.array(
        [(2 * rank + h_rel) * HD + dd
         for rank in range(NCORES) for h_rel in range(2) for dd in range(HD)]
    )
    heads_order = [2 * rank + h_rel
                   for rank in range(NCORES) for h_rel in range(2)]
    wout_t = np.ascontiguousarray(
        w_out[heads_order].reshape(NCORES * 128, D)
        .reshape(N_KB_D, 128, D).transpose(1, 0, 2)
    ).reshape(128, NCORES * D).astype(np.float16)

    def tile_w(Wc):          # [128 out, D kin] -> [128 p, (g, 128 out)]
        return np.ascontiguousarray(
            Wc.T.reshape(N_KB_D, 128, 128).transpose(1, 0, 2)
        ).reshape(128, D).astype(np.float16)

    in_maps = []
    for c in range(NCORES):
        heads = [2 * c, 2 * c + 1]
        W1b_c = W1b[128 * c:128 * (c + 1), :][:, k_idx]
        tA = [_tril_tiles(mat2a[h]) for h in heads]
        tB = [_tril_tiles(mat2b[h]) for h in heads]
        in_maps.append(
            {
                "xT_t": xT_t,
                "w1aT": tile_w(W1a[128 * c:128 * (c + 1), :]),
                "w1bT": tile_w(W1b_c),
                "trilAF": np.stack([t[0] for t in tA]),
                "trilAD": np.stack([t[1] for t in tA]),
                "trilBF": np.stack([t[0] for t in tB]),
                "trilBD": np.stack([t[1] for t in tB]),
                "wout_t": wout_t,
                "ident": ident,
            }
        )
    return in_maps


def kernel(x, W1a, W1b, mat2a, mat2b, w_out):
    x = np.asarray(x, dtype=np.float32)
    W1a = np.asarray(W1a, dtype=np.float32)
    W1b = np.asarray(W1b, dtype=np.float32)
    mat2a = np.asarray(mat2a, dtype=np.float32)
    mat2b = np.asarray(mat2b, dtype=np.float32)
    w_out = np.asarray(w_out, dtype=np.float32)

    if "nc" not in _NC_CACHE:
        _NC_CACHE["nc"] = build_nc()
    nc = _NC_CACHE["nc"]

    in_maps = prep_in_maps(x, W1a, W1b, mat2a, mat2b, w_out)
    res = run_bass_kernel_spmd(nc, in_maps, core_ids=list(range(NCORES)))
    out = np.concatenate(
        [res.results[c]["out_part"].astype(np.float32) for c in range(NCORES)],
        axis=0,
    )
    out *= 1.0 / OUT_SCALE
    return out.reshape(B, L, D)


if __name__ == "__main__":
    rng = np.random.default_rng(0)
    inputs = {
        "x": rng.standard_normal((B, L, D), dtype=np.float32),
        "W1a": rng.standard_normal((D, D), dtype=np.float32) / D,
        "W1b": rng.standard_normal((D, D), dtype=np.float32) / D,
        "mat2a": rng.standard_normal((H, L, L), dtype=np.float32) / 32,
        "mat2b": rng.standard_normal((H, L, L), dtype=np.float32) / 32,
        "w_out": rng.standard_normal((H, HD, D), dtype=np.float32) / D,
    }
    out = kernel(**inputs)
    print("kernel ran, out shape", out.shape)
